# revision 1
# baseline (speedup 1.0000x reference)
"""GCN-3 (2-layer bipartite GCN + MLP head) Trainium2 kernel, 8 NeuronCores.

Strategy (sharding_hint-aligned):
- Row-shard users/items (dest side) across 8 cores; edges partitioned by
  dest-row owner so each SpMM's segment-sum is core-local.
- SpMM per core: window-pure dma_gather (int16, 32K-row source windows) of
  256B rows -> val-scale (DVE) -> masked selection-matrix matmuls (PE) into
  PSUM -> accumulate into an SBUF-resident [128, C, 64] dest accumulator ->
  fused residual+relu (+ gcn combine + L2-partials on layer 2).
- Layer boundary: per-core shard outputs are host-concatenated (full tables
  re-uploaded for layer-2 gathers).
- Head (batch MLP/dot/MSE) runs data-parallel on the batch; per-core partial
  sums are combined on host (pure reduction glue).
- All structure (windows/cols/runs) is host-precomputed and PADDED so the
  8 cores share one SPMD instruction stream.
"""
import os
import sys

sys.path.insert(0, '/opt/trn_rl_repo')
os.environ.setdefault("MYCRO_LOCAL_CACHE", "1")

import numpy as np
import concourse.bacc as bacc
import concourse.bass as bass
import concourse.mybir as mybir
from concourse.tile import TileContext
from concourse.bass_utils import run_bass_kernel_spmd

F32 = mybir.dt.float32
I16 = mybir.dt.int16
AL = mybir.AluOpType

U = 359347
I = 292589
D = 64
B = 32768
LAM = 0.001

WIN = 32768
TSLOT = 1024
NPMAX = 24
NCORES = 8

_EXEC_NS = {"total": 0}   # filled per launch for test harness


# ----------------------------------------------------------------- planner --

def _side_mapping(n_orig):
    nd = int(np.ceil(n_orig / NCORES))
    C = int(np.ceil(nd / 128))
    rows_per_core = 128 * C
    n_pad = NCORES * rows_per_core
    ids = np.arange(n_orig)
    core = np.minimum(ids // nd, NCORES - 1)
    local = ids - core * nd
    storage = core * rows_per_core + (local % 128) * C + (local // 128)
    return dict(nd=nd, C=C, rows_per_core=rows_per_core, n_pad=n_pad,
                core=core.astype(np.int32), local=local.astype(np.int64),
                storage=storage.astype(np.int64))


def _plan_spmm(dest_core, dest_local, src_store, vals, C, ns_pad):
    nw = int(np.ceil(ns_pad / WIN))
    w = (src_store // WIN).astype(np.int64)
    col = (dest_local // 128).astype(np.int64)
    posv = (dest_local % 128).astype(np.float32)
    idx16 = (src_store % WIN).astype(np.int16)

    keys = (dest_core.astype(np.int64) * nw + w) * C + col
    cnt = np.bincount(keys, minlength=NCORES * nw * C).reshape(NCORES, nw, C)
    npad = np.maximum(cnt.max(axis=0), 1)

    cell_off = np.zeros((nw, C), np.int64)
    win_tiles = np.zeros(nw, np.int64)
    win_base = np.zeros(nw, np.int64)
    base = 0
    for wi in range(nw):
        co = np.cumsum(npad[wi])
        cell_off[wi, 1:] = co[:-1]
        t = (int(co[-1]) + TSLOT - 1) // TSLOT
        win_tiles[wi] = t
        win_base[wi] = base
        base += t * TSLOT
    total_slots = int(base)
    T = total_slots // TSLOT
    tile_window = np.zeros(T, np.int32)
    for wi in range(nw):
        t0 = win_base[wi] // TSLOT
        tile_window[t0:t0 + win_tiles[wi]] = wi

    slot_idx = np.zeros((NCORES, total_slots), np.int16)
    slot_val = np.zeros((NCORES, total_slots), np.float32)
    slot_pos = np.full((NCORES, total_slots), -10000.0, np.float32)

    order = np.lexsort((dest_local, w, dest_core))
    sk = keys[order]
    first = np.r_[True, sk[1:] != sk[:-1]]
    grp_start = np.flatnonzero(first)
    grp_id = np.cumsum(first) - 1
    rank = np.arange(len(sk)) - grp_start[grp_id]
    e_core = dest_core[order]
    slot = win_base[w[order]] + cell_off[w[order], col[order]] + rank
    slot_idx[e_core, slot] = idx16[order]
    slot_val[e_core, slot] = vals[order]
    slot_pos[e_core, slot] = posv[order]

    cell_npairs = {}
    for wi in range(nw):
        lo_ = win_base[wi] + cell_off[wi]
        hi_ = lo_ + npad[wi]
        for c in range(C):
            cell_npairs[(wi, c)] = (int(hi_[c]) - 1) // 128 - int(lo_[c]) // 128 + 1

    pairs_all, adds_all = [], []
    posq = np.full((NCORES, T, 128, NPMAX), -30000.0, np.float32)
    seen = {}
    for t in range(T):
        wi = int(tile_window[t])
        t0, t1 = t * TSLOT, (t + 1) * TSLOT
        lo_ = win_base[wi] + cell_off[wi]
        hi_ = lo_ + npad[wi]
        c_lo = int(np.searchsorted(hi_, t0, side='right'))
        c_hi = int(np.searchsorted(lo_, t1, side='left'))
        pairs = []
        np_i = 0
        for c in range(c_lo, c_hi):
            lo = max(int(lo_[c]), t0)
            hi = min(int(hi_[c]), t1)
            if lo >= hi:
                continue
            for g in range((lo - t0) // 128, (hi - 1 - t0) // 128 + 1):
                glo = max(lo - t0, g * 128)
                ghi = min(hi - t0, (g + 1) * 128)
                assert np_i < NPMAX, f"NPMAX exceeded (tile {t})"
                k = (wi, c)
                seen[k] = seen.get(k, 0) + 1
                pairs.append((np_i, g, c, seen[k] == 1,
                              seen[k] == cell_npairs[k]))
                ks = np.arange(glo - g * 128, ghi - g * 128)
                sl = np.arange(t0 + glo, t0 + ghi)
                posq[:, t, ks, np_i] = np_i * 128 + slot_pos[:, sl]
                np_i += 1
        pairs_all.append(pairs)
        adds_all.append([])

    col_done = {}
    for t in range(T):
        for (np_i, g, c, st, sp) in pairs_all[t]:
            if sp:
                col_done[(int(tile_window[t]), c)] = t
    for wi in range(nw):
        for o0 in range(0, C, 8):
            dts = [col_done[(wi, c)] for c in range(o0, min(o0 + 8, C))
                   if (wi, c) in col_done]
            if dts:
                adds_all[max(dts)].append((o0, min(8, C - o0)))

    idx_t = slot_idx.reshape(NCORES, T, 64, 16).transpose(0, 1, 3, 2)
    idx_t = np.ascontiguousarray(np.tile(idx_t, (1, 1, 8, 1)))
    val_t = np.ascontiguousarray(
        slot_val.reshape(NCORES, T, 8, 128).transpose(0, 1, 3, 2))
    return dict(T=T, nw=nw, tile_window=tile_window, pairs=pairs_all,
                adds=adds_all, idx=idx_t, val=val_t,
                pos=np.ascontiguousarray(posq),
                win_lo=[wi * WIN for wi in range(nw)],
                win_hi=[min((wi + 1) * WIN, ns_pad) for wi in range(nw)], C=C)


# ----------------------------------------------------------------- builder --

def _build_side(plan, C, ns_pad, repeat=1):
    """One SpMM side NEFF: gather->scale->masked-S matmul->acc ->
    relu+resid -> g table; gcn combine + ssq partial."""
    T, NW = plan['T'], plan['nw']
    nc = bacc.Bacc(num_swdge_queues=4)
    src = nc.dram_tensor("src", [ns_pad, D], F32, kind="ExternalInput")
    idx_d = nc.dram_tensor("idx", [T, 128, 64], I16, kind="ExternalInput")
    pos_d = nc.dram_tensor("pos", [T, 128, NPMAX], F32, kind="ExternalInput")
    val_d = nc.dram_tensor("val", [T, 128, 8], F32, kind="ExternalInput")
    resid = nc.dram_tensor("resid", [128, C, D], F32, kind="ExternalInput")
    emb = nc.dram_tensor("emb", [128, C, D], F32, kind="ExternalInput")
    dvec = nc.dram_tensor("dvec", [128, C], F32, kind="ExternalInput")
    wbc = nc.dram_tensor("wbc", [128, 4], F32, kind="ExternalInput")
    g_out = nc.dram_tensor("g_out", [128, C, D], F32, kind="ExternalOutput")
    gcn_out = nc.dram_tensor("gcn_out", [128, C, D], F32, kind="ExternalOutput")
    stats = nc.dram_tensor("stats", [1, 2], F32, kind="ExternalOutput")

    iota_np = np.tile(np.arange(NPMAX * 128, dtype=np.float32), (128, 1))
    iota_dr = nc.inline_tensor(iota_np, name="iota_c")
    ones_np = np.ones((128, 1), np.float32)
    ones_dr = nc.inline_tensor(ones_np, name="ones_c")

    with TileContext(nc) as tc:
        with (
            tc.tile_pool(name="big", bufs=1) as bigp,
            tc.tile_pool(name="aux", bufs=2) as auxp,
            tc.tile_pool(name="gat", bufs=4) as gatp,
            tc.tile_pool(name="sS", bufs=2) as sSp,
            tc.tile_pool(name="fin", bufs=3) as finp,
            tc.tile_pool(name="ps", bufs=4, space="PSUM") as psp,
            tc.tile_pool(name="ps1", bufs=1, space="PSUM") as ps1p,
        ):
            iota_t = bigp.tile([128, NPMAX * 128], F32, tag='iota', name='iota')
            nc.sync.dma_start(out=iota_t[:], in_=iota_dr[:])
            ones_t = bigp.tile([128, 1], F32, tag='ones', name='ones')
            nc.sync.dma_start(out=ones_t[:], in_=ones_dr[:])
            wbc_t = bigp.tile([128, 4], F32, tag='wbct', name='wbct')
            nc.sync.dma_start(out=wbc_t[:], in_=wbc[:])
            acc = bigp.tile([128, C * D], F32, tag='acc', name='acc')
            nc.vector.memset(acc[:], 0.0)
            ssq = bigp.tile([128, 1], F32, tag='ssq', name='ssq')
            nc.vector.memset(ssq[:], 0.0)

            idx_sb = pos_sb = val_sb = None
            for _rep in range(repeat):
              live = {}
              for t in range(T):
                  j = t % 8
                  if j == 0:
                      nchunk = min(8, T - t)
                      idx_sb = auxp.tile([128, 8, 64], I16, tag="idx", name="idx")
                      pos_sb = auxp.tile([128, 8, NPMAX], F32, tag="pos", name="pos")
                      val_sb = auxp.tile([128, 8, 8], F32, tag="val", name="val")
                      nc.sync.dma_start(
                          out=idx_sb[:, :nchunk, :],
                          in_=idx_d[t:t + nchunk].rearrange("t p q -> p t q"))
                      nc.sync.dma_start(
                          out=pos_sb[:, :nchunk, :],
                          in_=pos_d[t:t + nchunk].rearrange("t p q -> p t q"))
                      nc.sync.dma_start(
                          out=val_sb[:, :nchunk, :],
                          in_=val_d[t:t + nchunk].rearrange("t p q -> p t q"))
                  wi = int(plan['tile_window'][t])
                  g_t = gatp.tile([128, 8, D], F32, tag="g", name="g")
                  nc.gpsimd.dma_gather(
                      g_t[:], src[plan['win_lo'][wi]:plan['win_hi'][wi], :],
                      idx_sb[:, j, :], TSLOT, TSLOT, D,
                      single_packet=False, queue_num=t % 4)
                  nc.vector.tensor_tensor(
                      out=g_t[:], in0=g_t[:],
                      in1=val_sb[:, j, :].unsqueeze(2).to_broadcast([128, 8, D]),
                      op=AL.mult)
                  npairs = len(plan['pairs'][t])
                  S_t = sSp.tile([128, NPMAX * 128], F32, tag="S", name="S")
                  nc.vector.tensor_tensor(
                      out=S_t[:, :npairs * 128].rearrange(
                          "p (a b) -> p a b", b=128),
                      in0=pos_sb[:, j, :npairs].unsqueeze(2).to_broadcast(
                          [128, npairs, 128]),
                      in1=iota_t[:, :npairs * 128].rearrange(
                          "p (a b) -> p a b", b=128),
                      op=AL.is_equal)
                  for (np_i, gg, c, st, sp) in plan['pairs'][t]:
                      o = c // 8
                      if o not in live:
                          live[o] = psp.tile([128, 512], F32, tag="pb", name="pb")
                      nc.tensor.matmul(
                          out=live[o][:, (c % 8) * D:(c % 8 + 1) * D],
                          lhsT=S_t[:, np_i * 128:(np_i + 1) * 128],
                          rhs=g_t[:, gg, :],
                          start=st, stop=sp, skip_group_check=True)
                  for (c0, ncols) in plan['adds'][t]:
                      pt = live.pop(c0 // 8)
                      sl = acc[:, c0 * D:(c0 + ncols) * D]
                      nc.vector.tensor_tensor(out=sl, in0=sl,
                                              in1=pt[:, :ncols * D], op=AL.add)

            # final: g = relu(acc + resid*d); gcn = w0*emb+w1*resid+w2*g; ssq
            for c0 in range(0, C, 8):
                k = min(8, C - c0)
                r_sb = finp.tile([128, 8, D], F32, tag="r", name="r")
                e_sb = finp.tile([128, 8, D], F32, tag="e", name="e")
                d_sb = finp.tile([128, 8], F32, tag="d", name="d")
                nc.sync.dma_start(out=r_sb[:, :k, :], in_=resid[:, c0:c0 + k, :])
                nc.sync.dma_start(out=e_sb[:, :k, :], in_=emb[:, c0:c0 + k, :])
                nc.sync.dma_start(out=d_sb[:, :k], in_=dvec[:, c0:c0 + k])
                rd = finp.tile([128, 8, D], F32, tag="rd", name="rd")
                nc.vector.tensor_tensor(
                    out=rd[:, :k, :], in0=r_sb[:, :k, :],
                    in1=d_sb[:, :k].unsqueeze(2).to_broadcast([128, k, D]),
                    op=AL.mult)
                gsl = acc[:, c0 * D:(c0 + k) * D].rearrange(
                    "p (a b) -> p a b", b=D)
                nc.vector.tensor_tensor(out=rd[:, :k, :], in0=rd[:, :k, :],
                                        in1=gsl, op=AL.add)
                g_sb = finp.tile([128, 8, D], F32, tag="gf", name="gf")
                nc.vector.tensor_scalar(
                    out=g_sb[:, :k, :], in0=rd[:, :k, :],
                    scalar1=0.0, scalar2=None, op0=AL.max)
                nc.sync.dma_start(out=g_out[:, c0:c0 + k, :], in_=g_sb[:, :k, :])
                # gcn
                t2 = finp.tile([128, 8, D], F32, tag="t2", name="t2")
                nc.vector.tensor_scalar(out=t2[:, :k, :], in0=e_sb[:, :k, :],
                                        scalar1=wbc_t[:, 0:1], scalar2=None,
                                        op0=AL.mult)
                t3 = finp.tile([128, 8, D], F32, tag="t3", name="t3")
                nc.vector.tensor_scalar(out=t3[:, :k, :], in0=r_sb[:, :k, :],
                                        scalar1=wbc_t[:, 1:2], scalar2=None,
                                        op0=AL.mult)
                nc.vector.tensor_tensor(out=t2[:, :k, :], in0=t2[:, :k, :],
                                        in1=t3[:, :k, :], op=AL.add)
                nc.vector.tensor_scalar(out=t3[:, :k, :], in0=g_sb[:, :k, :],
                                        scalar1=wbc_t[:, 2:3], scalar2=None,
                                        op0=AL.mult)
                nc.vector.tensor_tensor(out=t2[:, :k, :], in0=t2[:, :k, :],
                                        in1=t3[:, :k, :], op=AL.add)
                nc.sync.dma_start(out=gcn_out[:, c0:c0 + k, :],
                                  in_=t2[:, :k, :])
                nc.vector.tensor_tensor(out=t3[:, :k, :], in0=t2[:, :k, :],
                                        in1=t2[:, :k, :], op=AL.mult)
                part = finp.tile([128, 1], F32, tag="pp", name="pp")
                nc.vector.tensor_reduce(out=part[:], in_=t3[:, :k, :],
                                        axis=mybir.AxisListType.XY, op=AL.add)
                nc.vector.tensor_tensor(out=ssq[:], in0=ssq[:], in1=part[:],
                                        op=AL.add)

            sq_ps = ps1p.tile([1, 2], F32, space="PSUM", tag='sqps', name='sqps')
            nc.tensor.matmul(out=sq_ps[:1, 0:1], lhsT=ones_t[:],
                             rhs=ssq[:], start=True, stop=True)
            st_sb = finp.tile([1, 2], F32, tag="st", name="st")
            nc.vector.memset(st_sb[:], 0.0)
            nc.vector.tensor_copy(out=st_sb[:1, 0:1], in_=sq_ps[:1, 0:1])
            nc.sync.dma_start(out=stats[:], in_=st_sb[:])
    nc.finalize()
    return nc


def _build_head(nb, repeat=1):
    """Batch head: leaky-MLP on user/item gcn rows, dot, + biases, sse."""
    nc = bacc.Bacc()
    xu = nc.dram_tensor("xu", [D, nb], F32, kind="ExternalInput")
    xi = nc.dram_tensor("xi", [D, nb], F32, kind="ExternalInput")
    fw1t = nc.dram_tensor("fw1t", [D, 2 * D], F32, kind="ExternalInput")
    fb1 = nc.dram_tensor("fb1", [2 * D, 1], F32, kind="ExternalInput")
    fw2t = nc.dram_tensor("fw2t", [2 * D, D], F32, kind="ExternalInput")
    fb2 = nc.dram_tensor("fb2", [D, 1], F32, kind="ExternalInput")
    bsum = nc.dram_tensor("bsum", [1, nb], F32, kind="ExternalInput")
    rat = nc.dram_tensor("rat", [1, nb], F32, kind="ExternalInput")
    out = nc.dram_tensor("out", [1, 1], F32, kind="ExternalOutput")
    ones_dr = nc.inline_tensor(np.ones((D, 1), np.float32), name="ones_h")

    with TileContext(nc) as tc:
        with (
            tc.tile_pool(name="sb", bufs=1) as sp,
            tc.tile_pool(name="wk", bufs=2) as wk,
            tc.tile_pool(name="ps", bufs=2, space="PSUM") as psp,
        ):
            xu_t = sp.tile([D, nb], F32, tag='xu', name='xu')
            xi_t = sp.tile([D, nb], F32, tag='xi', name='xi')
            w1 = sp.tile([D, 2 * D], F32, tag='w1', name='w1')
            b1 = sp.tile([2 * D, 1], F32, tag='b1', name='b1')
            w2 = sp.tile([2 * D, D], F32, tag='w2', name='w2')
            b2 = sp.tile([D, 1], F32, tag='b2', name='b2')
            on = sp.tile([D, 1], F32, tag='on', name='on')
            bs = sp.tile([1, nb], F32, tag='bs', name='bs')
            rt = sp.tile([1, nb], F32, tag='rt', name='rt')
            for t_, d_ in [(xu_t, xu), (xi_t, xi), (w1, fw1t), (b1, fb1),
                           (w2, fw2t), (b2, fb2), (on, ones_dr), (bs, bsum),
                           (rt, rat)]:
                nc.sync.dma_start(out=t_[:], in_=d_[:])

            for _rep in range(repeat):
                outs = []
                for (xt, side) in [(xu_t, 0), (xi_t, 1)]:
                    h_all = sp.tile([2 * D, nb], F32, tag=f"h{side}")
                    for n0 in range(0, nb, 512):
                        nn = min(512, nb - n0)
                        hp = psp.tile([128, 512], F32, tag="hp", space="PSUM")
                        nc.tensor.matmul(out=hp[:, :nn], lhsT=w1[:],
                                         rhs=xt[:, n0:n0 + nn],
                                         start=True, stop=True)
                        sl = h_all[:, n0:n0 + nn]
                        nc.vector.tensor_scalar(out=sl, in0=hp[:, :nn],
                                                scalar1=b1[:, 0:1], scalar2=None,
                                                op0=AL.add)
                        t_ = wk.tile([2 * D, 512], F32, tag="lk", name="lk")
                        nc.vector.tensor_scalar(out=t_[:, :nn], in0=sl, scalar1=0.1,
                                                scalar2=None, op0=AL.mult)
                        nc.vector.tensor_tensor(out=sl, in0=sl, in1=t_[:, :nn],
                                                op=AL.max)
                    o_all = sp.tile([D, nb], F32, tag=f"o{side}")
                    for n0 in range(0, nb, 512):
                        nn = min(512, nb - n0)
                        op_ = psp.tile([D, 512], F32, tag="op", space="PSUM")
                        nc.tensor.matmul(out=op_[:, :nn], lhsT=w2[:],
                                         rhs=h_all[:, n0:n0 + nn],
                                         start=True, stop=True)
                        sl = o_all[:, n0:n0 + nn]
                        nc.vector.tensor_scalar(out=sl, in0=op_[:, :nn],
                                                scalar1=b2[:, 0:1], scalar2=None,
                                                op0=AL.add)
                        t_ = wk.tile([D, 512], F32, tag="lk2", name="lk2")
                        nc.vector.tensor_scalar(out=t_[:, :nn], in0=sl, scalar1=0.1,
                                                scalar2=None, op0=AL.mult)
                        nc.vector.tensor_tensor(out=sl, in0=sl, in1=t_[:, :nn],
                                                op=AL.max)
                    outs.append(o_all)

                prod = sp.tile([D, nb], F32, tag='prod', name='prod')
                nc.vector.tensor_tensor(out=prod[:], in0=outs[0][:],
                                        in1=outs[1][:], op=AL.mult)
                pred = sp.tile([1, nb], F32, tag='pred', name='pred')
                for n0 in range(0, nb, 512):
                    nn = min(512, nb - n0)
                    pp = psp.tile([1, 512], F32, tag="pp", space="PSUM")
                    nc.tensor.matmul(out=pp[:1, :nn], lhsT=on[:],
                                     rhs=prod[:, n0:n0 + nn],
                                     start=True, stop=True)
                    nc.vector.tensor_copy(out=pred[:, n0:n0 + nn], in_=pp[:1, :nn])
                nc.vector.tensor_tensor(out=pred[:], in0=pred[:], in1=bs[:],
                                        op=AL.add)
                nc.vector.tensor_tensor(out=pred[:], in0=pred[:], in1=rt[:],
                                        op=AL.subtract)
                nc.vector.tensor_tensor(out=pred[:], in0=pred[:], in1=pred[:],
                                        op=AL.mult)
                sse = sp.tile([1, 1], F32, tag='sse', name='sse')
                nc.vector.tensor_reduce(out=sse[:], in_=pred[:],
                                        axis=mybir.AxisListType.X, op=AL.add)
                nc.sync.dma_start(out=out[:], in_=sse[:])
    nc.finalize()
    return nc


# ------------------------------------------------------------ orchestration --

def _to_storage_tables(arr, mp):
    """orig [n, D] -> storage [n_pad, D]."""
    out = np.zeros((mp['n_pad'], arr.shape[1]), np.float32)
    out[mp['storage']] = arr
    return out


def _shard_3d(full, mp, core):
    """storage-flat [n_pad, D] -> core view [128, C, D]."""
    C = mp['C']
    blk = full[core * mp['rows_per_core']:(core + 1) * mp['rows_per_core']]
    return np.ascontiguousarray(blk.reshape(128, C, -1))


def _run(nc, in_maps, label):
    import time
    t0 = time.time()
    res = run_bass_kernel_spmd(nc, in_maps, core_ids=list(range(len(in_maps))))
    wall = time.time() - t0
    _EXEC_NS.setdefault("walls", []).append((label, wall))
    _EXEC_NS.setdefault("launches", []).append((label, nc, in_maps))
    return res.results


def kernel(ui_rows, ui_cols, ui_vals, iu_vals, d_i, d_j,
           embed_user, embed_item, add_w, fw1, fb1, fw2, fb2,
           user_bias, item_bias, avg_rating, user0, item_i0, ratings):
    ui_rows = np.asarray(ui_rows)
    ui_cols = np.asarray(ui_cols)
    mu = _side_mapping(U)
    mi = _side_mapping(I)

    # plans (edge structure shared by both layers)
    planA = _plan_spmm(mu['core'][ui_rows], mu['local'][ui_rows],
                       mi['storage'][ui_cols], np.asarray(ui_vals, np.float32),
                       mu['C'], mi['n_pad'])
    planB = _plan_spmm(mi['core'][ui_cols], mi['local'][ui_cols],
                       mu['storage'][ui_rows], np.asarray(iu_vals, np.float32),
                       mi['C'], mu['n_pad'])

    ncA = _build_side(planA, mu['C'], mi['n_pad'])
    ncB = _build_side(planB, mi['C'], mu['n_pad'])
    _EXEC_NS['sideinfo'] = [("A", planA, mu['C'], mi['n_pad']),
                            ("B", planB, mi['C'], mu['n_pad'])]

    eu_st = _to_storage_tables(np.asarray(embed_user, np.float32), mu)
    ei_st = _to_storage_tables(np.asarray(embed_item, np.float32), mi)
    du_st = np.zeros(mu['n_pad'], np.float32)
    du_st[mu['storage']] = np.asarray(d_i, np.float32)
    dj_st = np.zeros(mi['n_pad'], np.float32)
    dj_st[mi['storage']] = np.asarray(d_j, np.float32)
    w = np.asarray(add_w, np.float32)[0]
    wbc = np.tile(np.r_[w, 0.0].astype(np.float32), (128, 1))

    def side_maps(plan, mp_d, src_full, resid_full, emb_full, d_full):
        maps = []
        for c in range(NCORES):
            C = mp_d['C']
            maps.append({
                "src": src_full,
                "idx": plan['idx'][c], "pos": plan['pos'][c],
                "val": plan['val'][c],
                "resid": _shard_3d(resid_full, mp_d, c),
                "emb": _shard_3d(emb_full, mp_d, c),
                "dvec": np.ascontiguousarray(
                    d_full[c * mp_d['rows_per_core']:
                           (c + 1) * mp_d['rows_per_core']].reshape(128, C)),
                "wbc": wbc,
            })
        return maps

    def collect_table(results):
        return np.concatenate(
            [r["g_out"].reshape(-1, D) for r in results], axis=0)

    def collect_gcn(results):
        return np.concatenate(
            [r["gcn_out"].reshape(-1, D) for r in results], axis=0)

    # layer 1
    rA1 = _run(ncA, side_maps(planA, mu, ei_st, eu_st, eu_st, du_st), "A1")
    g1u = collect_table(rA1)
    rB1 = _run(ncB, side_maps(planB, mi, eu_st, ei_st, ei_st, dj_st), "B1")
    g1i = collect_table(rB1)
    # layer 2
    rA2 = _run(ncA, side_maps(planA, mu, g1i, g1u, eu_st, du_st), "A2")
    gcnu = collect_gcn(rA2)
    ssq_u = sum(float(r["stats"][0, 0]) for r in rA2)
    rB2 = _run(ncB, side_maps(planB, mi, g1u, g1i, ei_st, dj_st), "B2")
    gcni = collect_gcn(rB2)
    ssq_i = sum(float(r["stats"][0, 0]) for r in rB2)

    # head: host gathers batch rows (pure indexing), device does the math
    nb = B // NCORES
    user0 = np.asarray(user0)
    item_i0 = np.asarray(item_i0)
    xu_rows = gcnu[mu['storage'][user0]]          # [B, D]
    xi_rows = gcni[mi['storage'][item_i0]]
    bsum = (np.asarray(user_bias, np.float32)[user0, 0]
            + np.asarray(item_bias, np.float32)[item_i0, 0]
            + np.float32(np.asarray(avg_rating, np.float32)[0]))
    nch = _build_head(nb)
    _EXEC_NS['headnb'] = nb
    hmaps = []
    for c in range(NCORES):
        sl = slice(c * nb, (c + 1) * nb)
        hmaps.append({
            "xu": np.ascontiguousarray(xu_rows[sl].T),
            "xi": np.ascontiguousarray(xi_rows[sl].T),
            "fw1t": np.ascontiguousarray(np.asarray(fw1, np.float32).T),
            "fb1": np.asarray(fb1, np.float32).reshape(2 * D, 1),
            "fw2t": np.ascontiguousarray(np.asarray(fw2, np.float32).T),
            "fb2": np.asarray(fb2, np.float32).reshape(D, 1),
            "bsum": bsum[sl].reshape(1, nb),
            "rat": np.asarray(ratings, np.float32)[sl].reshape(1, nb),
        })
    rH = _run(nch, hmaps, "H")
    sse = sum(float(r["out"][0, 0]) for r in rH)

    loss = (sse / B + LAM * ssq_u / (U * D) + LAM * ssq_i / (I * D))
    return np.float32(loss)



# revision 3
# speedup vs baseline: 127.7627x; 127.7627x over previous
"""GCN-3 (2-layer bipartite GCN + MLP head) Trainium2 kernel, 8 NeuronCores.

Strategy (sharding_hint-aligned):
- Row-shard users/items (dest side) across 8 cores; edges partitioned by
  dest-row owner so each SpMM's segment-sum is core-local.
- SpMM per core: single-packet dma_gather (int16, 32K-row source windows) of
  256B rows -> val-scale to bf16 (ACT) -> masked selection-matrix bf16
  matmuls (PE, FWL) into PSUM -> accumulate into an SBUF-resident
  [128, C, 64] dest accumulator -> fused residual+relu (+ gcn combine +
  L2-partials on layer 2).
- Layer boundary: per-core shard outputs are host-concatenated (full tables
  re-uploaded for layer-2 gathers).
- Head (batch MLP/dot/MSE) runs data-parallel on the batch; per-core partial
  sums are combined on host (pure reduction glue).
- All structure (windows/cols/runs) is host-precomputed and PADDED so the
  8 cores share one SPMD instruction stream.
"""
import os
import sys

sys.path.insert(0, '/opt/trn_rl_repo')
os.environ.setdefault("MYCRO_LOCAL_CACHE", "1")

import numpy as np
import ml_dtypes
import concourse.bacc as bacc
import concourse.bass as bass
import concourse.mybir as mybir
from concourse.tile import TileContext
from concourse.bass_utils import run_bass_kernel_spmd

F32 = mybir.dt.float32
BF16 = mybir.dt.bfloat16
I16 = mybir.dt.int16
AL = mybir.AluOpType
BF = ml_dtypes.bfloat16

U = 359347
I = 292589
D = 64
B = 32768
LAM = 0.001

WIN = 32768
TSLOT = 1024
NPMAX = 24
NCORES = 8

_EXEC_NS = {"total": 0}   # filled per launch for test harness


# ----------------------------------------------------------------- planner --

def _side_mapping(n_orig):
    nd = int(np.ceil(n_orig / NCORES))
    C = int(np.ceil(nd / 128))
    rows_per_core = 128 * C
    n_pad = NCORES * rows_per_core
    ids = np.arange(n_orig)
    core = np.minimum(ids // nd, NCORES - 1)
    local = ids - core * nd
    storage = core * rows_per_core + (local % 128) * C + (local // 128)
    return dict(nd=nd, C=C, rows_per_core=rows_per_core, n_pad=n_pad,
                core=core.astype(np.int32), local=local.astype(np.int64),
                storage=storage.astype(np.int64))


def _plan_spmm(dest_core, dest_local, src_store, vals, C, ns_pad):
    nw = int(np.ceil(ns_pad / WIN))
    w = (src_store // WIN).astype(np.int64)
    col = (dest_local // 128).astype(np.int64)
    posv = (dest_local % 128).astype(np.float32)
    idx16 = (src_store % WIN).astype(np.int16)

    keys = (dest_core.astype(np.int64) * nw + w) * C + col
    cnt = np.bincount(keys, minlength=NCORES * nw * C).reshape(NCORES, nw, C)
    npad = np.maximum(cnt.max(axis=0), 1)

    cell_off = np.zeros((nw, C), np.int64)
    win_tiles = np.zeros(nw, np.int64)
    win_base = np.zeros(nw, np.int64)
    base = 0
    for wi in range(nw):
        co = np.cumsum(npad[wi])
        cell_off[wi, 1:] = co[:-1]
        t = (int(co[-1]) + TSLOT - 1) // TSLOT
        win_tiles[wi] = t
        win_base[wi] = base
        base += t * TSLOT
    total_slots = int(base)
    T = total_slots // TSLOT
    tile_window = np.zeros(T, np.int32)
    for wi in range(nw):
        t0 = win_base[wi] // TSLOT
        tile_window[t0:t0 + win_tiles[wi]] = wi

    slot_idx = np.zeros((NCORES, total_slots), np.int16)
    slot_val = np.zeros((NCORES, total_slots), np.float32)
    slot_pos = np.full((NCORES, total_slots), -10000.0, np.float32)

    order = np.lexsort((dest_local, w, dest_core))
    sk = keys[order]
    first = np.r_[True, sk[1:] != sk[:-1]]
    grp_start = np.flatnonzero(first)
    grp_id = np.cumsum(first) - 1
    rank = np.arange(len(sk)) - grp_start[grp_id]
    e_core = dest_core[order]
    slot = win_base[w[order]] + cell_off[w[order], col[order]] + rank
    slot_idx[e_core, slot] = idx16[order]
    slot_val[e_core, slot] = vals[order]
    slot_pos[e_core, slot] = posv[order]

    cell_npairs = {}
    for wi in range(nw):
        lo_ = win_base[wi] + cell_off[wi]
        hi_ = lo_ + npad[wi]
        for c in range(C):
            cell_npairs[(wi, c)] = (int(hi_[c]) - 1) // 128 - int(lo_[c]) // 128 + 1

    pairs_all, adds_all = [], []
    posq = np.full((NCORES, T, 128, NPMAX), -10000.0, np.float32)
    seen = {}
    for t in range(T):
        wi = int(tile_window[t])
        t0, t1 = t * TSLOT, (t + 1) * TSLOT
        lo_ = win_base[wi] + cell_off[wi]
        hi_ = lo_ + npad[wi]
        c_lo = int(np.searchsorted(hi_, t0, side='right'))
        c_hi = int(np.searchsorted(lo_, t1, side='left'))
        pairs = []
        np_i = 0
        for c in range(c_lo, c_hi):
            lo = max(int(lo_[c]), t0)
            hi = min(int(hi_[c]), t1)
            if lo >= hi:
                continue
            for g in range((lo - t0) // 128, (hi - 1 - t0) // 128 + 1):
                glo = max(lo - t0, g * 128)
                ghi = min(hi - t0, (g + 1) * 128)
                assert np_i < NPMAX, f"NPMAX exceeded (tile {t})"
                k = (wi, c)
                seen[k] = seen.get(k, 0) + 1
                pairs.append((np_i, g, c, seen[k] == 1,
                              seen[k] == cell_npairs[k]))
                ks = np.arange(glo - g * 128, ghi - g * 128)
                sl = np.arange(t0 + glo, t0 + ghi)
                posq[:, t, ks, np_i] = slot_pos[:, sl]
                np_i += 1
        pairs_all.append(pairs)
        adds_all.append([])

    col_done = {}
    for t in range(T):
        for (np_i, g, c, st, sp) in pairs_all[t]:
            if sp:
                col_done[(int(tile_window[t]), c)] = t
    for wi in range(nw):
        for o0 in range(0, C, 8):
            dts = [col_done[(wi, c)] for c in range(o0, min(o0 + 8, C))
                   if (wi, c) in col_done]
            if dts:
                adds_all[max(dts)].append((o0, min(8, C - o0)))

    idx_t = slot_idx.reshape(NCORES, T, 64, 16).transpose(0, 1, 3, 2)
    idx_t = np.ascontiguousarray(np.tile(idx_t, (1, 1, 8, 1)))
    val_t = np.ascontiguousarray(
        slot_val.reshape(NCORES, T, 8, 128).transpose(0, 1, 3, 2))
    return dict(T=T, nw=nw, tile_window=tile_window, pairs=pairs_all,
                adds=adds_all, idx=idx_t, val=val_t,
                pos=np.ascontiguousarray(posq).astype(BF),
                win_lo=[wi * WIN for wi in range(nw)],
                win_hi=[min((wi + 1) * WIN, ns_pad) for wi in range(nw)], C=C)


# ----------------------------------------------------------------- builder --

def _build_side(plan, C, ns_pad, repeat=1, layer=2):
    """One SpMM side NEFF: gather->scale->masked-S matmul->acc.
    layer=1: out = g table (relu(acc + resid*d)) only.
    layer=2: out = gcn combine + ssq stats only."""
    T, NW = plan['T'], plan['nw']
    nc = bacc.Bacc(num_swdge_queues=4)
    src = nc.dram_tensor("src", [ns_pad, D], F32, kind="ExternalInput")
    idx_d = nc.dram_tensor("idx", [T, 128, 64], I16, kind="ExternalInput")
    pos_d = nc.dram_tensor("pos", [T, 128, NPMAX], BF16, kind="ExternalInput")
    val_d = nc.dram_tensor("val", [T, 128, 8], F32, kind="ExternalInput")
    resid = nc.dram_tensor("resid", [128, C, D], F32, kind="ExternalInput")
    dvec = nc.dram_tensor("dvec", [128, C], F32, kind="ExternalInput")
    if layer == 1:
        g_out = nc.dram_tensor("g_out", [128, C, D], F32, kind="ExternalOutput")
    else:
        emb = nc.dram_tensor("emb", [128, C, D], F32, kind="ExternalInput")
        wbc = nc.dram_tensor("wbc", [128, 4], F32, kind="ExternalInput")
        gcn_out = nc.dram_tensor("gcn_out", [128, C, D], F32,
                                 kind="ExternalOutput")
        stats = nc.dram_tensor("stats", [1, 2], F32, kind="ExternalOutput")

    iota_np = np.tile(np.arange(128, dtype=np.float32), (128, 1)).astype(BF)
    iota_dr = nc.inline_tensor(iota_np, name="iota_c")
    ones_np = np.ones((128, 1), np.float32)
    ones_dr = nc.inline_tensor(ones_np, name="ones_c")

    with TileContext(nc) as tc:
        with (
            tc.tile_pool(name="big", bufs=1) as bigp,
            tc.tile_pool(name="aux", bufs=2) as auxp,
            tc.tile_pool(name="gat", bufs=4) as gatp,
            tc.tile_pool(name="gb", bufs=4) as gbp,
            tc.tile_pool(name="sS", bufs=2) as sSp,
            tc.tile_pool(name="fin", bufs=3) as finp,
            tc.tile_pool(name="ps", bufs=4, space="PSUM") as psp,
            tc.tile_pool(name="ps1", bufs=1, space="PSUM") as ps1p,
        ):
            iota_t = bigp.tile([128, 128], BF16, tag='iota', name='iota')
            nc.sync.dma_start(out=iota_t[:], in_=iota_dr[:])
            acc = bigp.tile([128, C * D], F32, tag='acc', name='acc')
            nc.vector.memset(acc[:], 0.0)
            if layer == 2:
                ones_t = bigp.tile([128, 1], F32, tag='ones', name='ones')
                nc.sync.dma_start(out=ones_t[:], in_=ones_dr[:])
                wbc_t = bigp.tile([128, 4], F32, tag='wbct', name='wbct')
                nc.sync.dma_start(out=wbc_t[:], in_=wbc[:])
                ssq = bigp.tile([128, 1], F32, tag='ssq', name='ssq')
                nc.vector.memset(ssq[:], 0.0)

            idx_sb = pos_sb = val_sb = None
            for _rep in range(repeat):
              live = {}
              for t in range(T):
                  j = t % 8
                  if j == 0:
                      nchunk = min(8, T - t)
                      idx_sb = auxp.tile([128, 8, 64], I16, tag="idx", name="idx")
                      pos_sb = auxp.tile([128, 8, NPMAX], BF16, tag="pos",
                                         name="pos")
                      val_sb = auxp.tile([128, 8, 8], F32, tag="val", name="val")
                      nc.sync.dma_start(
                          out=idx_sb[:, :nchunk, :],
                          in_=idx_d[t:t + nchunk].rearrange("t p q -> p t q"))
                      nc.sync.dma_start(
                          out=pos_sb[:, :nchunk, :],
                          in_=pos_d[t:t + nchunk].rearrange("t p q -> p t q"))
                      nc.sync.dma_start(
                          out=val_sb[:, :nchunk, :],
                          in_=val_d[t:t + nchunk].rearrange("t p q -> p t q"))
                  wi = int(plan['tile_window'][t])
                  g_t = gatp.tile([128, 8, D], F32, tag="g", name="g")
                  nc.gpsimd.dma_gather(
                      g_t[:], src[plan['win_lo'][wi]:plan['win_hi'][wi], :],
                      idx_sb[:, j, :], TSLOT, TSLOT, D,
                      single_packet=True, queue_num=t % 4)
                  g_b = gbp.tile([128, 8, D], BF16, tag="gb", name="gb")
                  for gg in range(8):
                      nc.scalar.mul(out=g_b[:, gg, :], in_=g_t[:, gg, :],
                                    mul=val_sb[:, j, gg:gg + 1])
                  npairs = len(plan['pairs'][t])
                  S_t = sSp.tile([128, NPMAX * 128], BF16, tag="S", name="S")
                  nc.vector.tensor_tensor(
                      out=S_t[:, :npairs * 128].rearrange(
                          "p (a b) -> p a b", b=128),
                      in0=pos_sb[:, j, :npairs].unsqueeze(2).to_broadcast(
                          [128, npairs, 128]),
                      in1=iota_t[:].unsqueeze(1).to_broadcast(
                          [128, npairs, 128]),
                      op=AL.is_equal)
                  for (np_i, gg, c, st, sp) in plan['pairs'][t]:
                      o = c // 8
                      if o not in live:
                          live[o] = psp.tile([128, 512], F32, tag="pb", name="pb")
                      nc.tensor.matmul(
                          out=live[o][:, (c % 8) * D:(c % 8 + 1) * D],
                          lhsT=S_t[:, np_i * 128:(np_i + 1) * 128],
                          rhs=g_b[:, gg, :],
                          start=st, stop=sp, skip_group_check=True)
                  for (c0, ncols) in plan['adds'][t]:
                      pt = live.pop(c0 // 8)
                      sl = acc[:, c0 * D:(c0 + ncols) * D]
                      nc.vector.tensor_tensor(out=sl, in0=sl,
                                              in1=pt[:, :ncols * D], op=AL.add)

            # final: g = relu(acc + resid*d); layer2: gcn = w0*emb+w1*resid
            # +w2*g; ssq(gcn)
            for c0 in range(0, C, 8):
                k = min(8, C - c0)
                r_sb = finp.tile([128, 8, D], F32, tag="r", name="r")
                d_sb = finp.tile([128, 8], F32, tag="d", name="d")
                nc.sync.dma_start(out=r_sb[:, :k, :], in_=resid[:, c0:c0 + k, :])
                nc.sync.dma_start(out=d_sb[:, :k], in_=dvec[:, c0:c0 + k])
                rd = finp.tile([128, 8, D], F32, tag="rd", name="rd")
                nc.vector.tensor_tensor(
                    out=rd[:, :k, :], in0=r_sb[:, :k, :],
                    in1=d_sb[:, :k].unsqueeze(2).to_broadcast([128, k, D]),
                    op=AL.mult)
                gsl = acc[:, c0 * D:(c0 + k) * D].rearrange(
                    "p (a b) -> p a b", b=D)
                nc.vector.tensor_tensor(out=rd[:, :k, :], in0=rd[:, :k, :],
                                        in1=gsl, op=AL.add)
                g_sb = finp.tile([128, 8, D], F32, tag="gf", name="gf")
                nc.vector.tensor_scalar(
                    out=g_sb[:, :k, :], in0=rd[:, :k, :],
                    scalar1=0.0, scalar2=None, op0=AL.max)
                if layer == 1:
                    nc.sync.dma_start(out=g_out[:, c0:c0 + k, :],
                                      in_=g_sb[:, :k, :])
                    continue
                e_sb = finp.tile([128, 8, D], F32, tag="e", name="e")
                nc.sync.dma_start(out=e_sb[:, :k, :], in_=emb[:, c0:c0 + k, :])
                t2 = finp.tile([128, 8, D], F32, tag="t2", name="t2")
                nc.vector.tensor_scalar(out=t2[:, :k, :], in0=e_sb[:, :k, :],
                                        scalar1=wbc_t[:, 0:1], scalar2=None,
                                        op0=AL.mult)
                t3 = finp.tile([128, 8, D], F32, tag="t3", name="t3")
                nc.vector.tensor_scalar(out=t3[:, :k, :], in0=r_sb[:, :k, :],
                                        scalar1=wbc_t[:, 1:2], scalar2=None,
                                        op0=AL.mult)
                nc.vector.tensor_tensor(out=t2[:, :k, :], in0=t2[:, :k, :],
                                        in1=t3[:, :k, :], op=AL.add)
                nc.vector.tensor_scalar(out=t3[:, :k, :], in0=g_sb[:, :k, :],
                                        scalar1=wbc_t[:, 2:3], scalar2=None,
                                        op0=AL.mult)
                nc.vector.tensor_tensor(out=t2[:, :k, :], in0=t2[:, :k, :],
                                        in1=t3[:, :k, :], op=AL.add)
                nc.sync.dma_start(out=gcn_out[:, c0:c0 + k, :],
                                  in_=t2[:, :k, :])
                nc.vector.tensor_tensor(out=t3[:, :k, :], in0=t2[:, :k, :],
                                        in1=t2[:, :k, :], op=AL.mult)
                part = finp.tile([128, 1], F32, tag="pp", name="pp")
                nc.vector.tensor_reduce(out=part[:], in_=t3[:, :k, :],
                                        axis=mybir.AxisListType.XY, op=AL.add)
                nc.vector.tensor_tensor(out=ssq[:], in0=ssq[:], in1=part[:],
                                        op=AL.add)

            if layer == 2:
                sq_ps = ps1p.tile([1, 2], F32, space="PSUM", tag='sqps',
                                  name='sqps')
                nc.tensor.matmul(out=sq_ps[:1, 0:1], lhsT=ones_t[:],
                                 rhs=ssq[:], start=True, stop=True)
                st_sb = finp.tile([1, 2], F32, tag="st", name="st")
                nc.vector.memset(st_sb[:], 0.0)
                nc.vector.tensor_copy(out=st_sb[:1, 0:1], in_=sq_ps[:1, 0:1])
                nc.sync.dma_start(out=stats[:], in_=st_sb[:])
    nc.finalize()
    return nc


def _build_head(nb, repeat=1):
    """Batch head: leaky-MLP on user/item gcn rows, dot, + biases, sse."""
    nc = bacc.Bacc()
    xu = nc.dram_tensor("xu", [D, nb], F32, kind="ExternalInput")
    xi = nc.dram_tensor("xi", [D, nb], F32, kind="ExternalInput")
    fw1t = nc.dram_tensor("fw1t", [D, 2 * D], F32, kind="ExternalInput")
    fb1 = nc.dram_tensor("fb1", [2 * D, 1], F32, kind="ExternalInput")
    fw2t = nc.dram_tensor("fw2t", [2 * D, D], F32, kind="ExternalInput")
    fb2 = nc.dram_tensor("fb2", [D, 1], F32, kind="ExternalInput")
    bsum = nc.dram_tensor("bsum", [1, nb], F32, kind="ExternalInput")
    rat = nc.dram_tensor("rat", [1, nb], F32, kind="ExternalInput")
    out = nc.dram_tensor("out", [1, 1], F32, kind="ExternalOutput")
    ones_dr = nc.inline_tensor(np.ones((D, 1), np.float32), name="ones_h")

    with TileContext(nc) as tc:
        with (
            tc.tile_pool(name="sb", bufs=1) as sp,
            tc.tile_pool(name="wk", bufs=2) as wk,
            tc.tile_pool(name="ps", bufs=2, space="PSUM") as psp,
        ):
            xu_t = sp.tile([D, nb], F32, tag='xu', name='xu')
            xi_t = sp.tile([D, nb], F32, tag='xi', name='xi')
            w1 = sp.tile([D, 2 * D], F32, tag='w1', name='w1')
            b1 = sp.tile([2 * D, 1], F32, tag='b1', name='b1')
            w2 = sp.tile([2 * D, D], F32, tag='w2', name='w2')
            b2 = sp.tile([D, 1], F32, tag='b2', name='b2')
            on = sp.tile([D, 1], F32, tag='on', name='on')
            bs = sp.tile([1, nb], F32, tag='bs', name='bs')
            rt = sp.tile([1, nb], F32, tag='rt', name='rt')
            for t_, d_ in [(xu_t, xu), (xi_t, xi), (w1, fw1t), (b1, fb1),
                           (w2, fw2t), (b2, fb2), (on, ones_dr), (bs, bsum),
                           (rt, rat)]:
                nc.sync.dma_start(out=t_[:], in_=d_[:])

            for _rep in range(repeat):
                outs = []
                for (xt, side) in [(xu_t, 0), (xi_t, 1)]:
                    h_all = sp.tile([2 * D, nb], F32, tag=f"h{side}")
                    for n0 in range(0, nb, 512):
                        nn = min(512, nb - n0)
                        hp = psp.tile([128, 512], F32, tag="hp", space="PSUM")
                        nc.tensor.matmul(out=hp[:, :nn], lhsT=w1[:],
                                         rhs=xt[:, n0:n0 + nn],
                                         start=True, stop=True)
                        sl = h_all[:, n0:n0 + nn]
                        nc.vector.tensor_scalar(out=sl, in0=hp[:, :nn],
                                                scalar1=b1[:, 0:1], scalar2=None,
                                                op0=AL.add)
                        t_ = wk.tile([2 * D, 512], F32, tag="lk", name="lk")
                        nc.vector.tensor_scalar(out=t_[:, :nn], in0=sl, scalar1=0.1,
                                                scalar2=None, op0=AL.mult)
                        nc.vector.tensor_tensor(out=sl, in0=sl, in1=t_[:, :nn],
                                                op=AL.max)
                    o_all = sp.tile([D, nb], F32, tag=f"o{side}")
                    for n0 in range(0, nb, 512):
                        nn = min(512, nb - n0)
                        op_ = psp.tile([D, 512], F32, tag="op", space="PSUM")
                        nc.tensor.matmul(out=op_[:, :nn], lhsT=w2[:],
                                         rhs=h_all[:, n0:n0 + nn],
                                         start=True, stop=True)
                        sl = o_all[:, n0:n0 + nn]
                        nc.vector.tensor_scalar(out=sl, in0=op_[:, :nn],
                                                scalar1=b2[:, 0:1], scalar2=None,
                                                op0=AL.add)
                        t_ = wk.tile([D, 512], F32, tag="lk2", name="lk2")
                        nc.vector.tensor_scalar(out=t_[:, :nn], in0=sl, scalar1=0.1,
                                                scalar2=None, op0=AL.mult)
                        nc.vector.tensor_tensor(out=sl, in0=sl, in1=t_[:, :nn],
                                                op=AL.max)
                    outs.append(o_all)

                prod = sp.tile([D, nb], F32, tag='prod', name='prod')
                nc.vector.tensor_tensor(out=prod[:], in0=outs[0][:],
                                        in1=outs[1][:], op=AL.mult)
                pred = sp.tile([1, nb], F32, tag='pred', name='pred')
                for n0 in range(0, nb, 512):
                    nn = min(512, nb - n0)
                    pp = psp.tile([1, 512], F32, tag="pp", space="PSUM")
                    nc.tensor.matmul(out=pp[:1, :nn], lhsT=on[:],
                                     rhs=prod[:, n0:n0 + nn],
                                     start=True, stop=True)
                    nc.vector.tensor_copy(out=pred[:, n0:n0 + nn], in_=pp[:1, :nn])
                nc.vector.tensor_tensor(out=pred[:], in0=pred[:], in1=bs[:],
                                        op=AL.add)
                nc.vector.tensor_tensor(out=pred[:], in0=pred[:], in1=rt[:],
                                        op=AL.subtract)
                nc.vector.tensor_tensor(out=pred[:], in0=pred[:], in1=pred[:],
                                        op=AL.mult)
                sse = sp.tile([1, 1], F32, tag='sse', name='sse')
                nc.vector.tensor_reduce(out=sse[:], in_=pred[:],
                                        axis=mybir.AxisListType.X, op=AL.add)
                nc.sync.dma_start(out=out[:], in_=sse[:])
    nc.finalize()
    return nc


# ------------------------------------------------------------ orchestration --

def _to_storage_tables(arr, mp):
    """orig [n, D] -> storage [n_pad, D]."""
    out = np.zeros((mp['n_pad'], arr.shape[1]), np.float32)
    out[mp['storage']] = arr
    return out


def _shard_3d(full, mp, core):
    """storage-flat [n_pad, D] -> core view [128, C, D]."""
    C = mp['C']
    blk = full[core * mp['rows_per_core']:(core + 1) * mp['rows_per_core']]
    return np.ascontiguousarray(blk.reshape(128, C, -1))


def _run(nc, in_maps, label):
    import time
    t0 = time.time()
    res = run_bass_kernel_spmd(nc, in_maps, core_ids=list(range(len(in_maps))))
    wall = time.time() - t0
    _EXEC_NS.setdefault("walls", []).append((label, wall))
    _EXEC_NS.setdefault("launches", []).append((label, nc, in_maps))
    return res.results


def kernel(ui_rows, ui_cols, ui_vals, iu_vals, d_i, d_j,
           embed_user, embed_item, add_w, fw1, fb1, fw2, fb2,
           user_bias, item_bias, avg_rating, user0, item_i0, ratings):
    ui_rows = np.asarray(ui_rows)
    ui_cols = np.asarray(ui_cols)
    mu = _side_mapping(U)
    mi = _side_mapping(I)

    # plans (edge structure shared by both layers)
    planA = _plan_spmm(mu['core'][ui_rows], mu['local'][ui_rows],
                       mi['storage'][ui_cols], np.asarray(ui_vals, np.float32),
                       mu['C'], mi['n_pad'])
    planB = _plan_spmm(mi['core'][ui_cols], mi['local'][ui_cols],
                       mu['storage'][ui_rows], np.asarray(iu_vals, np.float32),
                       mi['C'], mu['n_pad'])

    ncA1 = _build_side(planA, mu['C'], mi['n_pad'], layer=1)
    ncB1 = _build_side(planB, mi['C'], mu['n_pad'], layer=1)
    ncA2 = _build_side(planA, mu['C'], mi['n_pad'], layer=2)
    ncB2 = _build_side(planB, mi['C'], mu['n_pad'], layer=2)
    _EXEC_NS['sideinfo'] = [("A1", planA, mu['C'], mi['n_pad'], 1),
                            ("B1", planB, mi['C'], mu['n_pad'], 1),
                            ("A2", planA, mu['C'], mi['n_pad'], 2),
                            ("B2", planB, mi['C'], mu['n_pad'], 2)]

    eu_st = _to_storage_tables(np.asarray(embed_user, np.float32), mu)
    ei_st = _to_storage_tables(np.asarray(embed_item, np.float32), mi)
    du_st = np.zeros(mu['n_pad'], np.float32)
    du_st[mu['storage']] = np.asarray(d_i, np.float32)
    dj_st = np.zeros(mi['n_pad'], np.float32)
    dj_st[mi['storage']] = np.asarray(d_j, np.float32)
    w = np.asarray(add_w, np.float32)[0]
    wbc = np.tile(np.r_[w, 0.0].astype(np.float32), (128, 1))

    def side_maps(plan, mp_d, src_full, resid_full, d_full, emb_full=None):
        maps = []
        for c in range(NCORES):
            C = mp_d['C']
            m = {
                "src": src_full,
                "idx": plan['idx'][c], "pos": plan['pos'][c],
                "val": plan['val'][c],
                "resid": _shard_3d(resid_full, mp_d, c),
                "dvec": np.ascontiguousarray(
                    d_full[c * mp_d['rows_per_core']:
                           (c + 1) * mp_d['rows_per_core']].reshape(128, C)),
            }
            if emb_full is not None:
                m["emb"] = _shard_3d(emb_full, mp_d, c)
                m["wbc"] = wbc
            maps.append(m)
        return maps

    def collect_table(results):
        return np.concatenate(
            [r["g_out"].reshape(-1, D) for r in results], axis=0)

    def collect_gcn(results):
        return np.concatenate(
            [r["gcn_out"].reshape(-1, D) for r in results], axis=0)

    # layer 1
    rA1 = _run(ncA1, side_maps(planA, mu, ei_st, eu_st, du_st), "A1")
    g1u = collect_table(rA1)
    rB1 = _run(ncB1, side_maps(planB, mi, eu_st, ei_st, dj_st), "B1")
    g1i = collect_table(rB1)
    # layer 2
    rA2 = _run(ncA2, side_maps(planA, mu, g1i, g1u, du_st, eu_st), "A2")
    gcnu = collect_gcn(rA2)
    ssq_u = sum(float(r["stats"][0, 0]) for r in rA2)
    rB2 = _run(ncB2, side_maps(planB, mi, g1u, g1i, dj_st, ei_st), "B2")
    gcni = collect_gcn(rB2)
    ssq_i = sum(float(r["stats"][0, 0]) for r in rB2)

    # head: host gathers batch rows (pure indexing), device does the math
    nb = B // NCORES
    user0 = np.asarray(user0)
    item_i0 = np.asarray(item_i0)
    xu_rows = gcnu[mu['storage'][user0]]          # [B, D]
    xi_rows = gcni[mi['storage'][item_i0]]
    bsum = (np.asarray(user_bias, np.float32)[user0, 0]
            + np.asarray(item_bias, np.float32)[item_i0, 0]
            + np.float32(np.asarray(avg_rating, np.float32)[0]))
    nch = _build_head(nb)
    _EXEC_NS['headnb'] = nb
    hmaps = []
    for c in range(NCORES):
        sl = slice(c * nb, (c + 1) * nb)
        hmaps.append({
            "xu": np.ascontiguousarray(xu_rows[sl].T),
            "xi": np.ascontiguousarray(xi_rows[sl].T),
            "fw1t": np.ascontiguousarray(np.asarray(fw1, np.float32).T),
            "fb1": np.asarray(fb1, np.float32).reshape(2 * D, 1),
            "fw2t": np.ascontiguousarray(np.asarray(fw2, np.float32).T),
            "fb2": np.asarray(fb2, np.float32).reshape(D, 1),
            "bsum": bsum[sl].reshape(1, nb),
            "rat": np.asarray(ratings, np.float32)[sl].reshape(1, nb),
        })
    rH = _run(nch, hmaps, "H")
    sse = sum(float(r["out"][0, 0]) for r in rH)

    loss = (sse / B + LAM * ssq_u / (U * D) + LAM * ssq_i / (I * D))
    return np.float32(loss)


# revision 5
# speedup vs baseline: 344.8020x; 2.6988x over previous
"""GCN-3 (2-layer bipartite GCN + MLP head) Trainium2 kernel, 8 NeuronCores.

Strategy (sharding_hint-aligned):
- Row-shard users/items (dest side) across 8 cores; edges partitioned by
  dest-row owner so each SpMM's segment-sum is core-local.
- SpMM per core: single-packet dma_gather (int16, 32K-row source windows) of
  256B rows -> val-scale to bf16 (ACT) -> masked selection-matrix bf16
  matmuls (PE, FWL) into PSUM -> accumulate into an SBUF-resident
  [128, C, 64] dest accumulator -> fused residual+relu (+ gcn combine +
  L2-partials on layer 2).
- Layer boundary: per-core shard outputs are host-concatenated (full tables
  re-uploaded for layer-2 gathers).
- Head (batch MLP/dot/MSE) runs data-parallel on the batch; per-core partial
  sums are combined on host (pure reduction glue).
- All structure (windows/cols/runs) is host-precomputed and PADDED so the
  8 cores share one SPMD instruction stream.
"""
import os
import sys

sys.path.insert(0, '/opt/trn_rl_repo')
os.environ.setdefault("MYCRO_LOCAL_CACHE", "1")

import numpy as np
import ml_dtypes
import concourse.bacc as bacc
import concourse.bass as bass
import concourse.mybir as mybir
from concourse.tile import TileContext
from concourse.bass_utils import run_bass_kernel_spmd

F32 = mybir.dt.float32
BF16 = mybir.dt.bfloat16
I16 = mybir.dt.int16
AL = mybir.AluOpType
BF = ml_dtypes.bfloat16

U = 359347
I = 292589
D = 64
B = 32768
LAM = 0.001

WIN = 32768
TSLOT = 1024
NPMAX = 24
NCORES = 8

_EXEC_NS = {"total": 0}   # filled per launch for test harness


# ----------------------------------------------------------------- planner --

def _side_mapping(n_orig):
    nd = int(np.ceil(n_orig / NCORES))
    C = int(np.ceil(nd / 128))
    rows_per_core = 128 * C
    n_pad = NCORES * rows_per_core
    ids = np.arange(n_orig)
    core = np.minimum(ids // nd, NCORES - 1)
    local = ids - core * nd
    storage = core * rows_per_core + (local % 128) * C + (local // 128)
    return dict(nd=nd, C=C, rows_per_core=rows_per_core, n_pad=n_pad,
                core=core.astype(np.int32), local=local.astype(np.int64),
                storage=storage.astype(np.int64))


def _plan_spmm(dest_core, dest_local, src_store, vals, C, ns_pad):
    nw = int(np.ceil(ns_pad / WIN))
    w = (src_store // WIN).astype(np.int64)
    col = (dest_local // 128).astype(np.int64)
    posv = (dest_local % 128).astype(np.float32)
    idx16 = (src_store % WIN).astype(np.int16)

    keys = (dest_core.astype(np.int64) * nw + w) * C + col
    cnt = np.bincount(keys, minlength=NCORES * nw * C).reshape(NCORES, nw, C)
    npad = np.maximum(cnt.max(axis=0), 1)

    cell_off = np.zeros((nw, C), np.int64)
    win_tiles = np.zeros(nw, np.int64)
    win_base = np.zeros(nw, np.int64)
    base = 0
    for wi in range(nw):
        co = np.cumsum(npad[wi])
        cell_off[wi, 1:] = co[:-1]
        t = (int(co[-1]) + TSLOT - 1) // TSLOT
        win_tiles[wi] = t
        win_base[wi] = base
        base += t * TSLOT
    total_slots = int(base)
    T = total_slots // TSLOT
    tile_window = np.zeros(T, np.int32)
    for wi in range(nw):
        t0 = win_base[wi] // TSLOT
        tile_window[t0:t0 + win_tiles[wi]] = wi

    slot_idx = np.zeros((NCORES, total_slots), np.int16)
    slot_val = np.zeros((NCORES, total_slots), np.float32)
    slot_pos = np.full((NCORES, total_slots), -10000.0, np.float32)

    order = np.lexsort((dest_local, w, dest_core))
    sk = keys[order]
    first = np.r_[True, sk[1:] != sk[:-1]]
    grp_start = np.flatnonzero(first)
    grp_id = np.cumsum(first) - 1
    rank = np.arange(len(sk)) - grp_start[grp_id]
    e_core = dest_core[order]
    slot = win_base[w[order]] + cell_off[w[order], col[order]] + rank
    slot_idx[e_core, slot] = idx16[order]
    slot_val[e_core, slot] = vals[order]
    slot_pos[e_core, slot] = posv[order]

    cell_npairs = {}
    for wi in range(nw):
        lo_ = win_base[wi] + cell_off[wi]
        hi_ = lo_ + npad[wi]
        for c in range(C):
            cell_npairs[(wi, c)] = (int(hi_[c]) - 1) // 128 - int(lo_[c]) // 128 + 1

    pairs_all, adds_all = [], []
    posq = np.full((NCORES, T, 128, NPMAX), -10000.0, np.float32)
    seen = {}
    for t in range(T):
        wi = int(tile_window[t])
        t0, t1 = t * TSLOT, (t + 1) * TSLOT
        lo_ = win_base[wi] + cell_off[wi]
        hi_ = lo_ + npad[wi]
        c_lo = int(np.searchsorted(hi_, t0, side='right'))
        c_hi = int(np.searchsorted(lo_, t1, side='left'))
        pairs = []
        np_i = 0
        for c in range(c_lo, c_hi):
            lo = max(int(lo_[c]), t0)
            hi = min(int(hi_[c]), t1)
            if lo >= hi:
                continue
            for g in range((lo - t0) // 128, (hi - 1 - t0) // 128 + 1):
                glo = max(lo - t0, g * 128)
                ghi = min(hi - t0, (g + 1) * 128)
                assert np_i < NPMAX, f"NPMAX exceeded (tile {t})"
                k = (wi, c)
                seen[k] = seen.get(k, 0) + 1
                pairs.append((np_i, g, c, seen[k] == 1,
                              seen[k] == cell_npairs[k]))
                ks = np.arange(glo - g * 128, ghi - g * 128)
                sl = np.arange(t0 + glo, t0 + ghi)
                posq[:, t, ks, np_i] = slot_pos[:, sl]
                np_i += 1
        pairs_all.append(pairs)
        adds_all.append([])

    col_done = {}
    for t in range(T):
        for (np_i, g, c, st, sp) in pairs_all[t]:
            if sp:
                col_done[(int(tile_window[t]), c)] = t
    for wi in range(nw):
        for o0 in range(0, C, 8):
            dts = [col_done[(wi, c)] for c in range(o0, min(o0 + 8, C))
                   if (wi, c) in col_done]
            if dts:
                adds_all[max(dts)].append((o0, min(8, C - o0)))

    idx_t = slot_idx.reshape(NCORES, T, 64, 16).transpose(0, 1, 3, 2)
    idx_t = np.ascontiguousarray(np.tile(idx_t, (1, 1, 8, 1)))
    val_t = np.ascontiguousarray(
        slot_val.reshape(NCORES, T, 8, 128).transpose(0, 1, 3, 2))
    return dict(T=T, nw=nw, tile_window=tile_window, pairs=pairs_all,
                adds=adds_all, idx=idx_t, val=val_t,
                pos=np.ascontiguousarray(posq).astype(BF),
                win_lo=[wi * WIN for wi in range(nw)],
                win_hi=[min((wi + 1) * WIN, ns_pad) for wi in range(nw)], C=C)


# ----------------------------------------------------------------- builder --

def _build_side(plan, C, ns_pad, repeat=1, layer=2):
    """One SpMM side NEFF: gather->scale->masked-S matmul->acc.
    layer=1: out = g table (relu(acc + resid*d)) only.
    layer=2: out = gcn combine + ssq stats only."""
    T, NW = plan['T'], plan['nw']
    nc = bacc.Bacc(num_swdge_queues=4)
    src = nc.dram_tensor("src", [ns_pad, D], F32, kind="ExternalInput")
    idx_d = nc.dram_tensor("idx", [T, 128, 64], I16, kind="ExternalInput")
    pos_d = nc.dram_tensor("pos", [T, 128, NPMAX], BF16, kind="ExternalInput")
    val_d = nc.dram_tensor("val", [T, 128, 8], F32, kind="ExternalInput")
    resid = nc.dram_tensor("resid", [128, C, D], F32, kind="ExternalInput")
    dvec = nc.dram_tensor("dvec", [128, C], F32, kind="ExternalInput")
    if layer == 1:
        g_out = nc.dram_tensor("g_out", [128, C, D], F32, kind="ExternalOutput")
    else:
        emb = nc.dram_tensor("emb", [128, C, D], F32, kind="ExternalInput")
        wbc = nc.dram_tensor("wbc", [128, 4], F32, kind="ExternalInput")
        gcn_out = nc.dram_tensor("gcn_out", [128, C, D], F32,
                                 kind="ExternalOutput")
        stats = nc.dram_tensor("stats", [1, 2], F32, kind="ExternalOutput")

    iota_np = np.tile(np.arange(128, dtype=np.float32),
                      (128, NPMAX)).astype(BF)
    iota_dr = nc.inline_tensor(iota_np, name="iota_c")
    ones_np = np.ones((128, 1), np.float32)
    ones_dr = nc.inline_tensor(ones_np, name="ones_c")

    with TileContext(nc) as tc:
        with (
            tc.tile_pool(name="big", bufs=1) as bigp,
            tc.tile_pool(name="aux", bufs=2) as auxp,
            tc.tile_pool(name="gat", bufs=4) as gatp,
            tc.tile_pool(name="gb", bufs=4) as gbp,
            tc.tile_pool(name="sS", bufs=2) as sSp,
            tc.tile_pool(name="fin", bufs=3) as finp,
            tc.tile_pool(name="ps", bufs=4, space="PSUM") as psp,
            tc.tile_pool(name="ps1", bufs=1, space="PSUM") as ps1p,
        ):
            iota_t = bigp.tile([128, NPMAX * 128], BF16, tag='iota',
                               name='iota')
            nc.sync.dma_start(out=iota_t[:], in_=iota_dr[:])
            acc = bigp.tile([128, C * D], F32, tag='acc', name='acc')
            nc.vector.memset(acc[:], 0.0)
            if layer == 2:
                ones_t = bigp.tile([128, 1], F32, tag='ones', name='ones')
                nc.sync.dma_start(out=ones_t[:], in_=ones_dr[:])
                wbc_t = bigp.tile([128, 4], F32, tag='wbct', name='wbct')
                nc.sync.dma_start(out=wbc_t[:], in_=wbc[:])
                ssq = bigp.tile([128, 1], F32, tag='ssq', name='ssq')
                nc.vector.memset(ssq[:], 0.0)

            idx_sb = pos_sb = val_sb = None
            for _rep in range(repeat):
              live = {}
              for t in range(T):
                  j = t % 8
                  if j == 0:
                      nchunk = min(8, T - t)
                      idx_sb = auxp.tile([128, 8, 64], I16, tag="idx", name="idx")
                      pos_sb = auxp.tile([128, 8, NPMAX], BF16, tag="pos",
                                         name="pos")
                      val_sb = auxp.tile([128, 8, 8], F32, tag="val", name="val")
                      nc.sync.dma_start(
                          out=idx_sb[:, :nchunk, :],
                          in_=idx_d[t:t + nchunk].rearrange("t p q -> p t q"))
                      nc.sync.dma_start(
                          out=pos_sb[:, :nchunk, :],
                          in_=pos_d[t:t + nchunk].rearrange("t p q -> p t q"))
                      nc.sync.dma_start(
                          out=val_sb[:, :nchunk, :],
                          in_=val_d[t:t + nchunk].rearrange("t p q -> p t q"))
                  wi = int(plan['tile_window'][t])
                  g_t = gatp.tile([128, 8, D], F32, tag="g", name="g")
                  nc.gpsimd.dma_gather(
                      g_t[:], src[plan['win_lo'][wi]:plan['win_hi'][wi], :],
                      idx_sb[:, j, :], TSLOT, TSLOT, D,
                      single_packet=True, queue_num=t % 4)
                  g_b = gbp.tile([128, 8, D], BF16, tag="gb", name="gb")
                  for gg in range(8):
                      nc.scalar.mul(out=g_b[:, gg, :], in_=g_t[:, gg, :],
                                    mul=val_sb[:, j, gg:gg + 1])
                  npairs = len(plan['pairs'][t])
                  S_t = sSp.tile([128, NPMAX * 128], BF16, tag="S", name="S")
                  nc.vector.tensor_tensor(
                      out=S_t[:, :npairs * 128].rearrange(
                          "p (a b) -> p a b", b=128),
                      in0=pos_sb[:, j, :npairs].unsqueeze(2).to_broadcast(
                          [128, npairs, 128]),
                      in1=iota_t[:, :npairs * 128].rearrange(
                          "p (a b) -> p a b", b=128),
                      op=AL.is_equal)
                  for (np_i, gg, c, st, sp) in plan['pairs'][t]:
                      o = c // 8
                      if o not in live:
                          live[o] = psp.tile([128, 512], F32, tag="pb", name="pb")
                      nc.tensor.matmul(
                          out=live[o][:, (c % 8) * D:(c % 8 + 1) * D],
                          lhsT=S_t[:, np_i * 128:(np_i + 1) * 128],
                          rhs=g_b[:, gg, :],
                          start=st, stop=sp, skip_group_check=True)
                  for (c0, ncols) in plan['adds'][t]:
                      pt = live.pop(c0 // 8)
                      sl = acc[:, c0 * D:(c0 + ncols) * D]
                      nc.vector.tensor_tensor(out=sl, in0=sl,
                                              in1=pt[:, :ncols * D], op=AL.add)

            # final: g = relu(acc + resid*d); layer2: gcn = w0*emb+w1*resid
            # +w2*g; ssq(gcn)
            for c0 in range(0, C, 8):
                k = min(8, C - c0)
                r_sb = finp.tile([128, 8, D], F32, tag="r", name="r")
                d_sb = finp.tile([128, 8], F32, tag="d", name="d")
                nc.sync.dma_start(out=r_sb[:, :k, :], in_=resid[:, c0:c0 + k, :])
                nc.sync.dma_start(out=d_sb[:, :k], in_=dvec[:, c0:c0 + k])
                rd = finp.tile([128, 8, D], F32, tag="rd", name="rd")
                nc.vector.tensor_tensor(
                    out=rd[:, :k, :], in0=r_sb[:, :k, :],
                    in1=d_sb[:, :k].unsqueeze(2).to_broadcast([128, k, D]),
                    op=AL.mult)
                gsl = acc[:, c0 * D:(c0 + k) * D].rearrange(
                    "p (a b) -> p a b", b=D)
                nc.vector.tensor_tensor(out=rd[:, :k, :], in0=rd[:, :k, :],
                                        in1=gsl, op=AL.add)
                g_sb = finp.tile([128, 8, D], F32, tag="gf", name="gf")
                nc.vector.tensor_scalar(
                    out=g_sb[:, :k, :], in0=rd[:, :k, :],
                    scalar1=0.0, scalar2=None, op0=AL.max)
                if layer == 1:
                    nc.sync.dma_start(out=g_out[:, c0:c0 + k, :],
                                      in_=g_sb[:, :k, :])
                    continue
                e_sb = finp.tile([128, 8, D], F32, tag="e", name="e")
                nc.sync.dma_start(out=e_sb[:, :k, :], in_=emb[:, c0:c0 + k, :])
                t2 = finp.tile([128, 8, D], F32, tag="t2", name="t2")
                nc.vector.tensor_scalar(out=t2[:, :k, :], in0=e_sb[:, :k, :],
                                        scalar1=wbc_t[:, 0:1], scalar2=None,
                                        op0=AL.mult)
                t3 = finp.tile([128, 8, D], F32, tag="t3", name="t3")
                nc.vector.tensor_scalar(out=t3[:, :k, :], in0=r_sb[:, :k, :],
                                        scalar1=wbc_t[:, 1:2], scalar2=None,
                                        op0=AL.mult)
                nc.vector.tensor_tensor(out=t2[:, :k, :], in0=t2[:, :k, :],
                                        in1=t3[:, :k, :], op=AL.add)
                nc.vector.tensor_scalar(out=t3[:, :k, :], in0=g_sb[:, :k, :],
                                        scalar1=wbc_t[:, 2:3], scalar2=None,
                                        op0=AL.mult)
                nc.vector.tensor_tensor(out=t2[:, :k, :], in0=t2[:, :k, :],
                                        in1=t3[:, :k, :], op=AL.add)
                nc.sync.dma_start(out=gcn_out[:, c0:c0 + k, :],
                                  in_=t2[:, :k, :])
                nc.vector.tensor_tensor(out=t3[:, :k, :], in0=t2[:, :k, :],
                                        in1=t2[:, :k, :], op=AL.mult)
                part = finp.tile([128, 1], F32, tag="pp", name="pp")
                nc.vector.tensor_reduce(out=part[:], in_=t3[:, :k, :],
                                        axis=mybir.AxisListType.XY, op=AL.add)
                nc.vector.tensor_tensor(out=ssq[:], in0=ssq[:], in1=part[:],
                                        op=AL.add)

            if layer == 2:
                sq_ps = ps1p.tile([1, 2], F32, space="PSUM", tag='sqps',
                                  name='sqps')
                nc.tensor.matmul(out=sq_ps[:1, 0:1], lhsT=ones_t[:],
                                 rhs=ssq[:], start=True, stop=True)
                st_sb = finp.tile([1, 2], F32, tag="st", name="st")
                nc.vector.memset(st_sb[:], 0.0)
                nc.vector.tensor_copy(out=st_sb[:1, 0:1], in_=sq_ps[:1, 0:1])
                nc.sync.dma_start(out=stats[:], in_=st_sb[:])
    nc.finalize()
    return nc


def _build_head(nb, repeat=1):
    """Batch head: leaky-MLP on user/item gcn rows, dot, + biases, sse."""
    nc = bacc.Bacc()
    xu = nc.dram_tensor("xu", [D, nb], F32, kind="ExternalInput")
    xi = nc.dram_tensor("xi", [D, nb], F32, kind="ExternalInput")
    fw1t = nc.dram_tensor("fw1t", [D, 2 * D], F32, kind="ExternalInput")
    fb1 = nc.dram_tensor("fb1", [2 * D, 1], F32, kind="ExternalInput")
    fw2t = nc.dram_tensor("fw2t", [2 * D, D], F32, kind="ExternalInput")
    fb2 = nc.dram_tensor("fb2", [D, 1], F32, kind="ExternalInput")
    bsum = nc.dram_tensor("bsum", [1, nb], F32, kind="ExternalInput")
    rat = nc.dram_tensor("rat", [1, nb], F32, kind="ExternalInput")
    out = nc.dram_tensor("out", [1, 1], F32, kind="ExternalOutput")
    ones_dr = nc.inline_tensor(np.ones((D, 1), np.float32), name="ones_h")

    with TileContext(nc) as tc:
        with (
            tc.tile_pool(name="sb", bufs=1) as sp,
            tc.tile_pool(name="wk", bufs=2) as wk,
            tc.tile_pool(name="ps", bufs=2, space="PSUM") as psp,
        ):
            xu_t = sp.tile([D, nb], F32, tag='xu', name='xu')
            xi_t = sp.tile([D, nb], F32, tag='xi', name='xi')
            w1 = sp.tile([D, 2 * D], F32, tag='w1', name='w1')
            b1 = sp.tile([2 * D, 1], F32, tag='b1', name='b1')
            w2 = sp.tile([2 * D, D], F32, tag='w2', name='w2')
            b2 = sp.tile([D, 1], F32, tag='b2', name='b2')
            on = sp.tile([D, 1], F32, tag='on', name='on')
            bs = sp.tile([1, nb], F32, tag='bs', name='bs')
            rt = sp.tile([1, nb], F32, tag='rt', name='rt')
            for t_, d_ in [(xu_t, xu), (xi_t, xi), (w1, fw1t), (b1, fb1),
                           (w2, fw2t), (b2, fb2), (on, ones_dr), (bs, bsum),
                           (rt, rat)]:
                nc.sync.dma_start(out=t_[:], in_=d_[:])

            for _rep in range(repeat):
                outs = []
                for (xt, side) in [(xu_t, 0), (xi_t, 1)]:
                    h_all = sp.tile([2 * D, nb], F32, tag=f"h{side}")
                    for n0 in range(0, nb, 512):
                        nn = min(512, nb - n0)
                        hp = psp.tile([128, 512], F32, tag="hp", space="PSUM")
                        nc.tensor.matmul(out=hp[:, :nn], lhsT=w1[:],
                                         rhs=xt[:, n0:n0 + nn],
                                         start=True, stop=True)
                        sl = h_all[:, n0:n0 + nn]
                        nc.vector.tensor_scalar(out=sl, in0=hp[:, :nn],
                                                scalar1=b1[:, 0:1], scalar2=None,
                                                op0=AL.add)
                        t_ = wk.tile([2 * D, 512], F32, tag="lk", name="lk")
                        nc.vector.tensor_scalar(out=t_[:, :nn], in0=sl, scalar1=0.1,
                                                scalar2=None, op0=AL.mult)
                        nc.vector.tensor_tensor(out=sl, in0=sl, in1=t_[:, :nn],
                                                op=AL.max)
                    o_all = sp.tile([D, nb], F32, tag=f"o{side}")
                    for n0 in range(0, nb, 512):
                        nn = min(512, nb - n0)
                        op_ = psp.tile([D, 512], F32, tag="op", space="PSUM")
                        nc.tensor.matmul(out=op_[:, :nn], lhsT=w2[:],
                                         rhs=h_all[:, n0:n0 + nn],
                                         start=True, stop=True)
                        sl = o_all[:, n0:n0 + nn]
                        nc.vector.tensor_scalar(out=sl, in0=op_[:, :nn],
                                                scalar1=b2[:, 0:1], scalar2=None,
                                                op0=AL.add)
                        t_ = wk.tile([D, 512], F32, tag="lk2", name="lk2")
                        nc.vector.tensor_scalar(out=t_[:, :nn], in0=sl, scalar1=0.1,
                                                scalar2=None, op0=AL.mult)
                        nc.vector.tensor_tensor(out=sl, in0=sl, in1=t_[:, :nn],
                                                op=AL.max)
                    outs.append(o_all)

                prod = sp.tile([D, nb], F32, tag='prod', name='prod')
                nc.vector.tensor_tensor(out=prod[:], in0=outs[0][:],
                                        in1=outs[1][:], op=AL.mult)
                pred = sp.tile([1, nb], F32, tag='pred', name='pred')
                for n0 in range(0, nb, 512):
                    nn = min(512, nb - n0)
                    pp = psp.tile([1, 512], F32, tag="pp", space="PSUM")
                    nc.tensor.matmul(out=pp[:1, :nn], lhsT=on[:],
                                     rhs=prod[:, n0:n0 + nn],
                                     start=True, stop=True)
                    nc.vector.tensor_copy(out=pred[:, n0:n0 + nn], in_=pp[:1, :nn])
                nc.vector.tensor_tensor(out=pred[:], in0=pred[:], in1=bs[:],
                                        op=AL.add)
                nc.vector.tensor_tensor(out=pred[:], in0=pred[:], in1=rt[:],
                                        op=AL.subtract)
                nc.vector.tensor_tensor(out=pred[:], in0=pred[:], in1=pred[:],
                                        op=AL.mult)
                sse = sp.tile([1, 1], F32, tag='sse', name='sse')
                nc.vector.tensor_reduce(out=sse[:], in_=pred[:],
                                        axis=mybir.AxisListType.X, op=AL.add)
                nc.sync.dma_start(out=out[:], in_=sse[:])
    nc.finalize()
    return nc


# ------------------------------------------------------------ orchestration --

def _to_storage_tables(arr, mp):
    """orig [n, D] -> storage [n_pad, D]."""
    out = np.zeros((mp['n_pad'], arr.shape[1]), np.float32)
    out[mp['storage']] = arr
    return out


def _shard_3d(full, mp, core):
    """storage-flat [n_pad, D] -> core view [128, C, D]."""
    C = mp['C']
    blk = full[core * mp['rows_per_core']:(core + 1) * mp['rows_per_core']]
    return np.ascontiguousarray(blk.reshape(128, C, -1))


def _run(nc, in_maps, label):
    import time
    t0 = time.time()
    res = run_bass_kernel_spmd(nc, in_maps, core_ids=list(range(len(in_maps))))
    wall = time.time() - t0
    _EXEC_NS.setdefault("walls", []).append((label, wall))
    _EXEC_NS.setdefault("launches", []).append((label, nc, in_maps))
    return res.results


def kernel(ui_rows, ui_cols, ui_vals, iu_vals, d_i, d_j,
           embed_user, embed_item, add_w, fw1, fb1, fw2, fb2,
           user_bias, item_bias, avg_rating, user0, item_i0, ratings):
    ui_rows = np.asarray(ui_rows)
    ui_cols = np.asarray(ui_cols)
    mu = _side_mapping(U)
    mi = _side_mapping(I)

    # plans (edge structure shared by both layers)
    planA = _plan_spmm(mu['core'][ui_rows], mu['local'][ui_rows],
                       mi['storage'][ui_cols], np.asarray(ui_vals, np.float32),
                       mu['C'], mi['n_pad'])
    planB = _plan_spmm(mi['core'][ui_cols], mi['local'][ui_cols],
                       mu['storage'][ui_rows], np.asarray(iu_vals, np.float32),
                       mi['C'], mu['n_pad'])

    ncA1 = _build_side(planA, mu['C'], mi['n_pad'], layer=1)
    ncB1 = _build_side(planB, mi['C'], mu['n_pad'], layer=1)
    ncA2 = _build_side(planA, mu['C'], mi['n_pad'], layer=2)
    ncB2 = _build_side(planB, mi['C'], mu['n_pad'], layer=2)
    _EXEC_NS['sideinfo'] = [("A1", planA, mu['C'], mi['n_pad'], 1),
                            ("B1", planB, mi['C'], mu['n_pad'], 1),
                            ("A2", planA, mu['C'], mi['n_pad'], 2),
                            ("B2", planB, mi['C'], mu['n_pad'], 2)]

    eu_st = _to_storage_tables(np.asarray(embed_user, np.float32), mu)
    ei_st = _to_storage_tables(np.asarray(embed_item, np.float32), mi)
    du_st = np.zeros(mu['n_pad'], np.float32)
    du_st[mu['storage']] = np.asarray(d_i, np.float32)
    dj_st = np.zeros(mi['n_pad'], np.float32)
    dj_st[mi['storage']] = np.asarray(d_j, np.float32)
    w = np.asarray(add_w, np.float32)[0]
    wbc = np.tile(np.r_[w, 0.0].astype(np.float32), (128, 1))

    def side_maps(plan, mp_d, src_full, resid_full, d_full, emb_full=None):
        maps = []
        for c in range(NCORES):
            C = mp_d['C']
            m = {
                "src": src_full,
                "idx": plan['idx'][c], "pos": plan['pos'][c],
                "val": plan['val'][c],
                "resid": _shard_3d(resid_full, mp_d, c),
                "dvec": np.ascontiguousarray(
                    d_full[c * mp_d['rows_per_core']:
                           (c + 1) * mp_d['rows_per_core']].reshape(128, C)),
            }
            if emb_full is not None:
                m["emb"] = _shard_3d(emb_full, mp_d, c)
                m["wbc"] = wbc
            maps.append(m)
        return maps

    def collect_table(results):
        return np.concatenate(
            [r["g_out"].reshape(-1, D) for r in results], axis=0)

    def collect_gcn(results):
        return np.concatenate(
            [r["gcn_out"].reshape(-1, D) for r in results], axis=0)

    # layer 1
    rA1 = _run(ncA1, side_maps(planA, mu, ei_st, eu_st, du_st), "A1")
    g1u = collect_table(rA1)
    rB1 = _run(ncB1, side_maps(planB, mi, eu_st, ei_st, dj_st), "B1")
    g1i = collect_table(rB1)
    # layer 2
    rA2 = _run(ncA2, side_maps(planA, mu, g1i, g1u, du_st, eu_st), "A2")
    gcnu = collect_gcn(rA2)
    ssq_u = sum(float(r["stats"][0, 0]) for r in rA2)
    rB2 = _run(ncB2, side_maps(planB, mi, g1u, g1i, dj_st, ei_st), "B2")
    gcni = collect_gcn(rB2)
    ssq_i = sum(float(r["stats"][0, 0]) for r in rB2)

    # head: host gathers batch rows (pure indexing), device does the math
    nb = B // NCORES
    user0 = np.asarray(user0)
    item_i0 = np.asarray(item_i0)
    xu_rows = gcnu[mu['storage'][user0]]          # [B, D]
    xi_rows = gcni[mi['storage'][item_i0]]
    bsum = (np.asarray(user_bias, np.float32)[user0, 0]
            + np.asarray(item_bias, np.float32)[item_i0, 0]
            + np.float32(np.asarray(avg_rating, np.float32)[0]))
    nch = _build_head(nb)
    _EXEC_NS['headnb'] = nb
    hmaps = []
    for c in range(NCORES):
        sl = slice(c * nb, (c + 1) * nb)
        hmaps.append({
            "xu": np.ascontiguousarray(xu_rows[sl].T),
            "xi": np.ascontiguousarray(xi_rows[sl].T),
            "fw1t": np.ascontiguousarray(np.asarray(fw1, np.float32).T),
            "fb1": np.asarray(fb1, np.float32).reshape(2 * D, 1),
            "fw2t": np.ascontiguousarray(np.asarray(fw2, np.float32).T),
            "fb2": np.asarray(fb2, np.float32).reshape(D, 1),
            "bsum": bsum[sl].reshape(1, nb),
            "rat": np.asarray(ratings, np.float32)[sl].reshape(1, nb),
        })
    rH = _run(nch, hmaps, "H")
    sse = sum(float(r["out"][0, 0]) for r in rH)

    loss = (sse / B + LAM * ssq_u / (U * D) + LAM * ssq_i / (I * D))
    return np.float32(loss)
